# revision 8
# baseline (speedup 1.0000x reference)
"""BertSelfAttention on 8 Trainium2 NeuronCores.

Sharding: data parallel over batch (B=2) x tensor parallel over heads
(16 heads -> 4 groups of 4). Core c handles batch c//4, heads 4*(c%4)..+4.
No collectives: each core produces a disjoint [256, 2048] output slice
(feature-major); the host transposes/concatenates.

Per-core program (identical on all cores, SPMD over data):
  inputs (host-prepped):
    xt    [2, 8, 128, 1024]  hidden_states[b].T, split [half, chunk] (bf16)
    wq/wk/wv [128, 8, 256] weight column slices, partition-major, bf16
             (wq,qb2 pre-scaled 1/8)
    qb2/kb2 [128, 2]    bias chunks (per-partition layout, f32)
    em    [128, 16]     exp(mask) chunks (f32), emb same in bf16
    em4   [128, 64]     exp(mask) x4 replicated (bf16)
    one1  [128, 1]      ones (bf16)
  output:
    out   [256, 2048]   context slice, feature-major (f32)

The attention mask is folded into V: softmax(s+m) @ V == (exp(s) @
diag(exp(m)) V) / (sum_k exp(s) exp(m)), so V rows are pre-scaled by
exp(mask) and the exp on the Activation engine needs no bias operand.

Design (trace-driven):
- ACT (exp) is the roofline: 128 x [128,1024] exps at ~1.12us = 143us.
  Span ~= first-exp time + ACT stream + tail, so the prologue streams
  inputs over 5 DMA queues (contiguous host layouts) and ~8 junk
  matmuls warm the PE HAM clock gate during the DMA wait, putting the
  first exp at ~8us instead of 28us.
- Context matmuls are column-tiled: two heads' [64,512] ctx tiles run
  concurrently in PE column groups (0,0)/(0,64), halving ctx PE time so
  total PE work sits ~20us under the ACT roofline and the exp stream
  never starves. The softmax denominators that previously rode the ctx
  matmul as a 65th row come from a DVE running sum over key tiles
  (bf16) finished by a tiny ones-column matmul partition reduction.
- The last block (1,3) keeps the legacy 65-row ctx (denominator
  in-matmul) so the critical tail after the final exp is short.
"""

import numpy as np

HIDDEN = 1024
HEADS = 16
HD = 64
B = 2
S = 2048
NCORES = 8
HPC = HEADS // 4  # heads per core = 4
WCOLS = HPC * HD  # 256 weight columns per core

_CACHE = {}


def _build_program():
    import concourse.bass as bass
    import concourse.bacc as bacc
    import concourse.tile as tile
    import concourse.mybir as mybir

    f32 = mybir.dt.float32
    bf16 = mybir.dt.bfloat16
    Exp = mybir.ActivationFunctionType.Exp
    MUL = mybir.AluOpType.mult
    ADD = mybir.AluOpType.add

    nc = bacc.Bacc("TRN2", target_bir_lowering=False, debug=False, num_devices=NCORES)

    xt_d = nc.dram_tensor("xt", [2, 8, 128, 1024], bf16, kind="ExternalInput")
    wq_d = nc.dram_tensor("wq", [128, 8, WCOLS], bf16, kind="ExternalInput")
    wk_d = nc.dram_tensor("wk", [128, 8, WCOLS], bf16, kind="ExternalInput")
    wv_d = nc.dram_tensor("wv", [128, 8, WCOLS], bf16, kind="ExternalInput")
    qb_d = nc.dram_tensor("qb2", [128, 2], f32, kind="ExternalInput")
    kb_d = nc.dram_tensor("kb2", [128, 2], f32, kind="ExternalInput")
    em_d = nc.dram_tensor("em", [128, 16], f32, kind="ExternalInput")
    emb_d = nc.dram_tensor("emb", [128, 16], bf16, kind="ExternalInput")
    em4_d = nc.dram_tensor("em4", [128, 64], bf16, kind="ExternalInput")
    one_d = nc.dram_tensor("one1", [128, 1], bf16, kind="ExternalInput")
    out_d = nc.dram_tensor("out", [WCOLS, S], f32, kind="ExternalOutput")

    with (
        tile.TileContext(nc) as tc,
        tc.tile_pool(name="main", bufs=1) as P,
        tc.tile_pool(name="att", bufs=3) as att,
        tc.tile_pool(name="nrm", bufs=2) as nrm,
        tc.tile_pool(name="dnm", bufs=2) as dnm,
        tc.tile_pool(name="pqp", bufs=1, space="PSUM") as PQ,
        tc.tile_pool(name="psp", bufs=2, space="PSUM") as PS,
        tc.tile_pool(name="pcp", bufs=3, space="PSUM") as PC,
    ):
        xt = [P.tile([128, S], bf16, name=f"xt{k}") for k in range(8)]
        wq_sb = P.tile([128, 8, WCOLS], bf16)
        wk_sb = P.tile([128, 8, WCOLS], bf16)
        wv_sb = P.tile([128, 8, WCOLS], bf16)
        q_sb = P.tile([128, 2, S], bf16)  # [feat(2 heads), pair, token]
        k_sb = P.tile([128, 2, S], bf16)
        v_sb = P.tile([128, 16, 4 * 65], bf16)  # [token, tile, 4*(em + 64 feats)]
        v_blk = v_sb.rearrange("p m (l c) -> p m l c", l=4)
        qkb = P.tile([128, 4], f32)
        qb_sb = qkb[:, 0:2]
        kb_sb = qkb[:, 2:4]
        em_sb = P.tile([128, 16], f32)
        emb_sb = P.tile([128, 16], bf16)
        em4_sb = P.tile([128, 64], bf16)
        one_sb = P.tile([128, 1], bf16)
        warm = P.tile([128, 640], bf16)

        # ---- input DMAs on the 3 HW queues (sync, gpsimd, scalar),
        # contiguous host layouts, ordered so the prologue-critical
        # pieces (wq/wk + xt half 0) land first, KORD-arrival-matched.
        # The scalar queue is done by ~6us, before the first exp.
        nc.sync.dma_start(out=wq_sb[:, 0:4, :], in_=wq_d.ap()[:, 0:4, :])
        nc.gpsimd.dma_start(out=wk_sb[:, 0:4, :], in_=wk_d.ap()[:, 0:4, :])
        nc.scalar.dma_start(out=wq_sb[:, 4:8, :], in_=wq_d.ap()[:, 4:8, :])
        nc.sync.dma_start(out=xt[0][:, 0:1024], in_=xt_d.ap()[0, 0])
        nc.gpsimd.dma_start(out=xt[4][:, 0:1024], in_=xt_d.ap()[0, 4])
        nc.scalar.dma_start(out=wk_sb[:, 4:8, :], in_=wk_d.ap()[:, 4:8, :])
        nc.sync.dma_start(out=xt[1][:, 0:1024], in_=xt_d.ap()[0, 1])
        nc.gpsimd.dma_start(out=xt[5][:, 0:1024], in_=xt_d.ap()[0, 5])
        nc.scalar.dma_start(out=xt[2][:, 0:1024], in_=xt_d.ap()[0, 2])
        nc.sync.dma_start(out=xt[3][:, 0:1024], in_=xt_d.ap()[0, 3])
        nc.gpsimd.dma_start(out=xt[7][:, 0:1024], in_=xt_d.ap()[0, 7])
        nc.scalar.dma_start(out=xt[6][:, 0:1024], in_=xt_d.ap()[0, 6])
        # lower priority: biases/masks, V weights, xt half 1
        nc.sync.dma_start(out=qb_sb, in_=qb_d.ap())
        nc.sync.dma_start(out=kb_sb, in_=kb_d.ap())
        nc.sync.dma_start(out=one_sb[:], in_=one_d.ap())
        nc.gpsimd.dma_start(out=em_sb[:], in_=em_d.ap())
        nc.gpsimd.dma_start(out=emb_sb[:], in_=emb_d.ap())
        nc.gpsimd.dma_start(out=em4_sb[:], in_=em4_d.ap())
        nc.sync.dma_start(out=wv_sb[:, 0:4, :], in_=wv_d.ap()[:, 0:4, :])
        nc.gpsimd.dma_start(out=wv_sb[:, 4:8, :], in_=wv_d.ap()[:, 4:8, :])
        nc.sync.dma_start(out=xt[0][:, 1024:2048], in_=xt_d.ap()[1, 0])
        nc.gpsimd.dma_start(out=xt[4][:, 1024:2048], in_=xt_d.ap()[1, 4])
        nc.sync.dma_start(out=xt[1][:, 1024:2048], in_=xt_d.ap()[1, 1])
        nc.gpsimd.dma_start(out=xt[5][:, 1024:2048], in_=xt_d.ap()[1, 5])
        nc.sync.dma_start(out=xt[2][:, 1024:2048], in_=xt_d.ap()[1, 2])
        nc.gpsimd.dma_start(out=xt[6][:, 1024:2048], in_=xt_d.ap()[1, 6])
        nc.sync.dma_start(out=xt[3][:, 1024:2048], in_=xt_d.ap()[1, 3])
        nc.gpsimd.dma_start(out=xt[7][:, 1024:2048], in_=xt_d.ap()[1, 7])

        # ---- PE warmup: ~3.5us of junk matmuls while DMA streams, so
        # the HAM clock gate reaches 8/8 before real work starts.
        nc.gpsimd.memset(warm[:], 0.0)
        for _ in range(8):
            wp = PQ.tile([128, 512], f32, tag="pq", name="warm")
            nc.tensor.matmul(
                wp[:], lhsT=warm[:, 0:128], rhs=warm[:, 128:640],
                start=True, stop=True,
            )

        # contraction chunks in DMA-arrival order
        KORD = [0, 4, 1, 5, 2, 6, 3, 7]

        def proj_unit(w_sb, b_sb, dst, mc, sp):
            # one [128 feats, 512 tokens] projection block: 8 matmuls + bias
            pq = PQ.tile([128, 512], f32, tag="pq")
            for i, k in enumerate(KORD):
                nc.tensor.matmul(
                    pq[:],
                    lhsT=w_sb[:, k, mc * 128 : mc * 128 + 128],
                    rhs=xt[k][:, sp * 512 : sp * 512 + 512],
                    start=(i == 0),
                    stop=(i == 7),
                )
            nc.vector.tensor_scalar_add(
                dst[:, mc, sp * 512 : sp * 512 + 512], pq[:], b_sb[:, mc : mc + 1]
            )

        def v_unit(mt):
            # V token tile [128 tokens, 256 feats] + bias row, scaled by exp(mask)
            pv = PQ.tile([128, 512], f32, tag="pq", name="pv")[:, 0:256]
            for i, k in enumerate(KORD):
                nc.tensor.matmul(
                    pv[:],
                    lhsT=xt[k][:, mt * 128 : mt * 128 + 128],
                    rhs=wv_sb[:, k, :],
                    start=(i == 0),
                    stop=(i == 7),
                )
            nc.vector.tensor_scalar_mul(
                v_blk[:, mt, :, 1:65],
                pv.rearrange("p (l c) -> p l c", l=4),
                em_sb[:, mt : mt + 1],
            )
            nc.vector.tensor_copy(v_blk[:, mt, :, 0], em4_sb[:, 4 * mt : 4 * mt + 4])

        Dps = {}

        def s_kt(p, sp, kt, expP):
            # S^T for key tile kt (both heads of pair p) + mask-free exp,
            # then fold this tile into the DVE running denominator sum.
            qs = sp * 512
            ps = PS.tile([128, 1024], f32, tag="ps")
            for h in range(2):
                rs = 64 * h
                nc.tensor.matmul(
                    ps[:, h * 512 : h * 512 + 512],
                    lhsT=k_sb[rs : rs + 64, p, kt * 128 : kt * 128 + 128],
                    rhs=q_sb[rs : rs + 64, p, qs : qs + 512],
                    start=True,
                    stop=True,
                )
            nc.scalar.activation(expP[:, kt, :], ps[:], Exp)
            if (p, sp) != (1, 3):
                if kt == 0:
                    Dps[(p, sp)] = dnm.tile([128, 1024], bf16, tag="dp", name="dp")
                    nc.vector.tensor_scalar_mul(
                        Dps[(p, sp)][:], expP[:, kt, :], em_sb[:, kt : kt + 1]
                    )
                else:
                    nc.vector.scalar_tensor_tensor(
                        Dps[(p, sp)][:],
                        expP[:, kt, :],
                        em_sb[:, kt : kt + 1],
                        Dps[(p, sp)][:],
                        MUL,
                        ADD,
                    )

        pcs = {}

        def cu_unit(p, sp, u, expP):
            # 2 col-tiled accumulation steps of ctx^T: both heads of the
            # pair run concurrently in PE column groups (0,0)/(0,64).
            key = (p, sp)
            if u == 0:
                pcs[key] = PC.tile([128, 512], f32, tag="pc", name="pc")
            pc = pcs[key]
            for j in range(2):
                kt = 2 * u + j
                for h in range(2):
                    lh = 2 * p + h
                    nc.tensor.matmul(
                        pc[64 * h : 64 * h + 64, :],
                        lhsT=v_sb[:, kt, 65 * lh + 1 : 65 * lh + 65],
                        rhs=expP[:, kt, h * 512 : h * 512 + 512],
                        start=(kt == 0),
                        stop=(kt == 15),
                        skip_group_check=True,
                    )

        def fin_unit(p, sp):
            # denominator partition-reduce + normalize + store for a
            # col-tiled block: d = ones^T @ Dp, ctx *= 1/d (broadcast).
            qs = sp * 512
            pc = pcs[(p, sp)]
            Dp = Dps[(p, sp)]
            ctxs = nrm.tile([128, 512], f32, tag="cts")
            for h in range(2):
                dps = PQ.tile([128, 512], f32, tag="pq", name="dps")
                nc.tensor.matmul(
                    dps[0:1, :],
                    lhsT=one_sb[:, 0:1],
                    rhs=Dp[:, h * 512 : h * 512 + 512],
                    start=True,
                    stop=True,
                )
                r = nrm.tile([1, 512], f32, tag=f"r{h}")
                nc.vector.reciprocal_approx_fast(r[0:1, :], dps[0:1, :])
                bc = nrm.tile([128, 512], f32, tag=f"bc{h}")
                nc.gpsimd.partition_broadcast(bc[:, :], r[0:1, :])
                nc.vector.tensor_mul(
                    ctxs[64 * h : 64 * h + 64, :],
                    pc[64 * h : 64 * h + 64, :],
                    bc[64 * h : 64 * h + 64, :],
                )
            eng = nc.sync if sp % 2 == 0 else nc.gpsimd
            eng.dma_start(
                out=out_d.ap()[128 * p : 128 * p + 128, qs : qs + 512],
                in_=ctxs[:, :],
            )

        def c65_unit(p, sp, half, u, expP):
            # legacy ctx with in-matmul denominator row (last block only)
            lh = 2 * p + half
            key = (p, sp, half)
            if u == 0:
                pcs[key] = PC.tile([128, 512], f32, tag="pc", name=f"pc65_{half}")
            pc = pcs[key]
            for j in range(2):
                kt = 2 * u + j
                nc.tensor.matmul(
                    pc[0:65, :],
                    lhsT=v_sb[:, kt, 65 * lh : 65 * lh + 65],
                    rhs=expP[:, kt, half * 512 : half * 512 + 512],
                    start=(kt == 0),
                    stop=(kt == 15),
                )
            if u == 7:
                qs = sp * 512
                ctxs = nrm.tile([65, 512], f32, tag="cts65")
                bc = nrm.tile([65, 512], f32, tag="bc65")
                for q in range(2):
                    cs = slice(q * 256, (q + 1) * 256)
                    nc.vector.reciprocal_approx_fast(ctxs[0:1, cs], pc[0:1, cs])
                    nc.gpsimd.partition_broadcast(bc[:, cs], ctxs[0:1, cs])
                    nc.vector.tensor_mul(ctxs[:, cs], pc[0:65, cs], bc[:, cs])
                    eng = nc.sync if q == 0 else nc.gpsimd
                    eng.dma_start(
                        out=out_d.ap()[
                            64 * lh : 64 * lh + 64, qs + cs.start : qs + cs.stop
                        ],
                        in_=ctxs[1:65, cs],
                    )

        # ---- prologue: minimal chain to the first exp
        proj_unit(wq_sb, qb_sb, q_sb, 0, 0)
        proj_unit(wk_sb, kb_sb, k_sb, 0, 0)

        def pq_u(p, sp):
            return ("proj", wq_sb, qb_sb, q_sb, p, sp)

        def pk_u(p, sp):
            return ("proj", wk_sb, kb_sb, k_sb, p, sp)

        # per-block filler schedule: {kt: [tokens emitted after s_kt(kt)]}
        def sched(*pairs):
            d = {}
            for kt, tok in pairs:
                d.setdefault(kt, []).append(tok)
            return d

        fillers = {
            (0, 0): sched(
                (0, pk_u(0, 1)), (2, ("v", 0)), (4, pk_u(0, 2)), (6, ("v", 1)),
                (7, ("v", 2)), (8, pk_u(0, 3)), (10, ("v", 3)), (12, pq_u(0, 1)),
                (14, ("v", 4)),
            ),
            (0, 1): sched(
                (0, ("v", 5)), (1, ("cu", 0, 0, 0)), (2, ("v", 6)),
                (3, ("cu", 0, 0, 1)), (4, ("v", 7)), (5, ("cu", 0, 0, 2)),
                (6, ("v", 8)), (7, ("cu", 0, 0, 3)), (8, ("v", 9)),
                (9, ("cu", 0, 0, 4)), (10, ("v", 10)), (12, ("v", 11)),
                (13, pq_u(0, 2)),
            ),
            (0, 2): sched(
                (0, ("v", 12)), (1, ("v", 13)), (2, ("cu", 0, 0, 5)),
                (3, ("cu", 0, 0, 6)), (4, ("v", 14)), (5, ("v", 15)),
                (6, ("cu", 0, 0, 7)), (7, ("fin", 0, 0)), (8, ("cu", 0, 1, 0)),
                (9, ("cu", 0, 1, 1)), (10, ("cu", 0, 1, 2)), (11, pq_u(0, 3)),
                (12, ("cu", 0, 1, 3)), (13, ("cu", 0, 1, 4)),
                (14, ("cu", 0, 1, 5)), (15, ("cu", 0, 1, 6)),
            ),
            (0, 3): sched(
                (0, ("cu", 0, 1, 7)), (1, ("fin", 0, 1)), (2, ("cu", 0, 2, 0)),
                (3, ("cu", 0, 2, 1)), (4, ("cu", 0, 2, 2)), (5, ("cu", 0, 2, 3)),
                (6, ("cu", 0, 2, 4)), (7, ("cu", 0, 2, 5)), (8, ("cu", 0, 2, 6)),
                (9, ("cu", 0, 2, 7)), (10, ("fin", 0, 2)), (12, pk_u(1, 0)),
                (14, pq_u(1, 0)),
            ),
            (1, 0): sched(
                (0, pk_u(1, 1)), (1, ("cu", 0, 3, 0)), (2, ("cu", 0, 3, 1)),
                (4, pk_u(1, 2)), (5, ("cu", 0, 3, 2)), (6, ("cu", 0, 3, 3)),
                (8, pk_u(1, 3)), (9, ("cu", 0, 3, 4)), (10, ("cu", 0, 3, 5)),
                (12, pq_u(1, 1)),
            ),
            (1, 1): sched(
                (0, ("cu", 0, 3, 6)), (1, ("cu", 0, 3, 7)), (2, ("fin", 0, 3)),
                (3, ("cu", 1, 0, 0)), (4, ("cu", 1, 0, 1)), (5, ("cu", 1, 0, 2)),
                (6, ("cu", 1, 0, 3)), (7, ("cu", 1, 0, 4)), (8, ("cu", 1, 0, 5)),
                (9, ("cu", 1, 0, 6)), (10, ("cu", 1, 0, 7)), (12, ("fin", 1, 0)),
                (13, pq_u(1, 2)),
            ),
            (1, 2): sched(
                (0, ("cu", 1, 1, 0)), (1, ("cu", 1, 1, 1)), (2, pq_u(1, 3)),
                (2, ("cu", 1, 1, 2)), (4, ("cu", 1, 1, 3)), (5, ("cu", 1, 1, 4)),
                (6, ("cu", 1, 1, 5)), (7, ("cu", 1, 1, 6)), (8, ("cu", 1, 1, 7)),
                (9, ("fin", 1, 1)),
            ),
            (1, 3): sched(
                (0, ("cu", 1, 2, 7)), (2, ("fin", 1, 2)),
            ),
        }

        blocks = [(p, sp) for p in (0, 1) for sp in range(4)]
        expPs = {}

        def emit(tok, _ep=None):
            if tok[0] == "v":
                v_unit(tok[1])
            elif tok[0] == "proj":
                _, w, b, dst, mc, sp = tok
                proj_unit(w, b, dst, mc, sp)
            elif tok[0] == "cu":
                _, cp, csp, u = tok
                cu_unit(cp, csp, u, expPs[(cp, csp)])
            elif tok[0] == "fin":
                fin_unit(tok[1], tok[2])

        for p, sp in blocks:
            expP = att.tile([128, 16, 1024], bf16, tag="expP")
            expPs[(p, sp)] = expP
            fill = fillers[(p, sp)]
            chase = (p, sp) in ((1, 2), (1, 3))
            for kt in range(16):
                s_kt(p, sp, kt, expP)
                for tok in fill.get(kt, ()):
                    emit(tok)
                if chase and kt >= 3 and kt % 2 == 1:
                    u = (kt - 3) // 2
                    if (p, sp) == (1, 2):
                        cu_unit(1, 2, u, expP)
                    else:
                        c65_unit(1, 3, 0, u, expP)
                        c65_unit(1, 3, 1, u, expP)

        # epilogue: the last context chunk (its exps just finished)
        for half in range(2):
            c65_unit(1, 3, half, 7, expPs[(1, 3)])

    nc.compile()
    return nc


def _get_program():
    if "nc" not in _CACHE:
        _CACHE["nc"] = _build_program()
    return _CACHE["nc"]


def _to_bf16(x):
    import ml_dtypes

    return np.asarray(x, np.float32).astype(ml_dtypes.bfloat16)


def _make_in_maps(hidden_states, attention_mask, q_w, q_b, k_w, k_b, v_w, v_b):
    hs = np.asarray(hidden_states, np.float32)
    am = np.asarray(attention_mask, np.float32)
    q_w = np.asarray(q_w, np.float32)
    k_w = np.asarray(k_w, np.float32)
    v_w = np.asarray(v_w, np.float32)
    q_b = np.asarray(q_b, np.float32)
    k_b = np.asarray(k_b, np.float32)
    v_b = np.asarray(v_b, np.float32)

    scale = np.float32(1.0 / np.sqrt(HD))
    ones1 = np.ones((128, 1), np.float32)

    in_maps = []
    for c in range(NCORES):
        b = c // 4
        hg = c % 4
        cols = slice(WCOLS * hg, WCOLS * hg + WCOLS)
        mask = am[b, 0, 0, :]  # [S]
        em = np.exp(mask.reshape(16, 128).T).astype(np.float32)  # [128, 16]
        em4 = np.repeat(em[:, :, None], 4, axis=2).reshape(128, 64)
        # xt: [1024, 2048] -> [half, chunk, 128, 1024]
        xt = hs[b].T.reshape(8, 128, 2, 1024).transpose(2, 0, 1, 3)

        def wlay(w):
            # [1024, 256] -> [128, 8, 256] (partition-major chunks)
            return np.ascontiguousarray(
                _to_bf16(w.reshape(8, 128, WCOLS).transpose(1, 0, 2))
            )

        in_maps.append(
            {
                "xt": np.ascontiguousarray(_to_bf16(xt)),
                "wq": wlay(q_w[:, cols] * scale),
                "wk": wlay(k_w[:, cols]),
                "wv": wlay(v_w[:, cols]),
                "qb2": np.ascontiguousarray((q_b[cols] * scale).reshape(2, 128).T),
                "kb2": np.ascontiguousarray(k_b[cols].reshape(2, 128).T),
                "em": np.ascontiguousarray(em),
                "emb": np.ascontiguousarray(_to_bf16(em)),
                "em4": np.ascontiguousarray(_to_bf16(em4)),
                "one1": np.ascontiguousarray(_to_bf16(ones1)),
            }
        )
    return in_maps


def kernel(hidden_states, attention_mask, q_w, q_b, k_w, k_b, v_w, v_b):
    from concourse import bass_utils

    nc = _get_program()
    in_maps = _make_in_maps(
        hidden_states, attention_mask, q_w, q_b, k_w, k_b, v_w, v_b
    )
    _CACHE["in_maps"] = in_maps
    res = bass_utils.run_bass_kernel_spmd(nc, in_maps, core_ids=list(range(NCORES)))

    full = np.empty((B, S, HIDDEN), np.float32)
    for c in range(NCORES):
        b = c // 4
        hg = c % 4
        full[b, :, WCOLS * hg : WCOLS * hg + WCOLS] = res.results[c]["out"].T
    # V bias contributes exactly v_b to every context vector (softmax
    # weights sum to 1), so it is added here instead of on-device.
    full += np.asarray(v_b, np.float32)[None, None, :]
    return full


# revision 9
# speedup vs baseline: 1.0752x; 1.0752x over previous
"""BertSelfAttention on 8 Trainium2 NeuronCores.

Sharding: data parallel over batch (B=2) x tensor parallel over heads
(16 heads -> 4 groups of 4). Core c handles batch c//4, heads 4*(c%4)..+4.
No collectives: each core produces a disjoint [256, 2048] output slice
(feature-major); the host transposes/concatenates.

Per-core program (identical on all cores, SPMD over data):
  inputs (host-prepped):
    xt    [2, 8, 128, 1024]  hidden_states[b].T, split [half, chunk] (bf16)
    wq/wk/wv [128, 8, 256] weight column slices, partition-major, bf16
             (wq,qb2 pre-scaled 1/8)
    qb2/kb2 [128, 2]    bias chunks (per-partition layout, f32)
    em    [128, 16]     exp(mask) chunks (f32), emb same in bf16
    em4   [128, 64]     exp(mask) x4 replicated (bf16)
    one1  [128, 1]      ones (bf16)
  output:
    out   [256, 2048]   context slice, feature-major (f32)

The attention mask is folded into V: softmax(s+m) @ V == (exp(s) @
diag(exp(m)) V) / (sum_k exp(s) exp(m)), so V rows are pre-scaled by
exp(mask) and the exp on the Activation engine needs no bias operand.

Design (trace-driven):
- ACT (exp) is the roofline: 128 x [128,1024] exps at ~1.12us = 143us.
  Span ~= first-exp time + ACT stream + tail, so the prologue streams
  inputs over 5 DMA queues (contiguous host layouts) and ~8 junk
  matmuls warm the PE HAM clock gate during the DMA wait, putting the
  first exp at ~8us instead of 28us.
- Context matmuls are column-tiled: two heads' [64,512] ctx tiles run
  concurrently in PE column groups (0,0)/(0,64), halving ctx PE time so
  total PE work sits ~20us under the ACT roofline and the exp stream
  never starves. The softmax denominators that previously rode the ctx
  matmul as a 65th row come from a DVE running sum over key tiles
  (bf16) finished by a tiny ones-column matmul partition reduction.
- The last block (1,3) keeps the legacy 65-row ctx (denominator
  in-matmul) so the critical tail after the final exp is short.
"""

import numpy as np

HIDDEN = 1024
HEADS = 16
HD = 64
B = 2
S = 2048
NCORES = 8
HPC = HEADS // 4  # heads per core = 4
WCOLS = HPC * HD  # 256 weight columns per core

_CACHE = {}


def _build_program():
    import concourse.bass as bass
    import concourse.bacc as bacc
    import concourse.tile as tile
    import concourse.mybir as mybir

    f32 = mybir.dt.float32
    bf16 = mybir.dt.bfloat16
    Exp = mybir.ActivationFunctionType.Exp
    MUL = mybir.AluOpType.mult
    ADD = mybir.AluOpType.add

    nc = bacc.Bacc("TRN2", target_bir_lowering=False, debug=False, num_devices=NCORES)

    xt_d = nc.dram_tensor("xt", [4, 8, 128, 512], bf16, kind="ExternalInput")
    wq_d = nc.dram_tensor("wq", [2, 128, 8, 128], bf16, kind="ExternalInput")
    wk_d = nc.dram_tensor("wk", [2, 128, 8, 128], bf16, kind="ExternalInput")
    wv_d = nc.dram_tensor("wv", [128, 8, WCOLS], bf16, kind="ExternalInput")
    qb_d = nc.dram_tensor("qb2", [128, 2], f32, kind="ExternalInput")
    kb_d = nc.dram_tensor("kb2", [128, 2], f32, kind="ExternalInput")
    mk_d = nc.dram_tensor("mk", [128, 16], f32, kind="ExternalInput")
    on4_d = nc.dram_tensor("ones4", [128, 64], bf16, kind="ExternalInput")
    one_d = nc.dram_tensor("one1", [128, 1], bf16, kind="ExternalInput")
    out_d = nc.dram_tensor("out", [WCOLS, S], f32, kind="ExternalOutput")

    with (
        tile.TileContext(nc) as tc,
        tc.tile_pool(name="main", bufs=1) as P,
        tc.tile_pool(name="att", bufs=3) as att,
        tc.tile_pool(name="nrm", bufs=2) as nrm,
        tc.tile_pool(name="dnm", bufs=2) as dnm,
        tc.tile_pool(name="pqp", bufs=1, space="PSUM") as PQ,
        tc.tile_pool(name="psp", bufs=2, space="PSUM") as PS,
        tc.tile_pool(name="pcp", bufs=3, space="PSUM") as PC,
    ):
        xt = [P.tile([128, S], bf16, name=f"xt{k}") for k in range(8)]
        wq_sb = P.tile([128, 8, WCOLS], bf16)
        wk_sb = P.tile([128, 8, WCOLS], bf16)
        wv_sb = P.tile([128, 8, WCOLS], bf16)
        q_sb = P.tile([128, 2, S], bf16)  # [feat(2 heads), pair, token]
        k_sb = P.tile([128, 2, S], bf16)
        v_sb = P.tile([128, 16, 4 * 65], bf16)  # [token, tile, 4*(em + 64 feats)]
        v_blk = v_sb.rearrange("p m (l c) -> p m l c", l=4)
        qkb = P.tile([128, 4], f32)
        qb_sb = qkb[:, 0:2]
        kb_sb = qkb[:, 2:4]
        mk_sb = P.tile([128, 16], f32)
        on4_sb = P.tile([128, 64], bf16)
        one_sb = P.tile([128, 1], bf16)
        warm = P.tile([128, 640], bf16)

        # ---- input DMAs on the 3 HW queues (sync, gpsimd, scalar).
        # Aggregate DMA BW is ~350GB/s with ~9us startup, so waves are
        # ordered by need: first-exp critical set is only wq/wk pair-0
        # halves + token-quarter 0 of xt (~1.5MB).
        # wave 1: biases, mask, mc0 weights, xt quarter 0
        nc.sync.dma_start(out=qb_sb, in_=qb_d.ap())
        nc.sync.dma_start(out=kb_sb, in_=kb_d.ap())
        nc.sync.dma_start(out=mk_sb[:], in_=mk_d.ap())
        nc.scalar.dma_start(out=wq_sb[:, :, 0:128], in_=wq_d.ap()[0])
        nc.gpsimd.dma_start(out=wk_sb[:, :, 0:128], in_=wk_d.ap()[0])
        for k in range(8):
            eng = (nc.sync, nc.gpsimd, nc.scalar)[k % 3]
            eng.dma_start(out=xt[k][:, 0:512], in_=xt_d.ap()[0, k])
        # wave 2: xt quarter 1 (keys 512-1023 by s4; Q(0,1) by kt12)
        for k in range(8):
            eng = (nc.sync, nc.gpsimd, nc.scalar)[k % 3]
            eng.dma_start(out=xt[k][:, 512:1024], in_=xt_d.ap()[1, k])
        # wave 3: xt quarter 2 + V weights (scalar queue stays clear now)
        for k in range(8):
            eng = nc.sync if k % 2 == 0 else nc.gpsimd
            eng.dma_start(out=xt[k][:, 1024:1536], in_=xt_d.ap()[2, k])
        nc.sync.dma_start(out=wv_sb[:, 0:4, :], in_=wv_d.ap()[:, 0:4, :])
        nc.gpsimd.dma_start(out=wv_sb[:, 4:8, :], in_=wv_d.ap()[:, 4:8, :])
        nc.gpsimd.dma_start(out=on4_sb[:], in_=on4_d.ap())
        nc.gpsimd.dma_start(out=one_sb[:], in_=one_d.ap())
        # wave 4: xt quarter 3
        for k in range(8):
            eng = nc.sync if k % 2 == 1 else nc.gpsimd
            eng.dma_start(out=xt[k][:, 1536:2048], in_=xt_d.ap()[3, k])
        # wave 5: mc1 weight halves (first needed ~70us in)
        nc.sync.dma_start(out=wq_sb[:, :, 128:256], in_=wq_d.ap()[1])
        nc.gpsimd.dma_start(out=wk_sb[:, :, 128:256], in_=wk_d.ap()[1])

        # ---- PE warmup: ~3.5us of junk matmuls while DMA streams, so
        # the HAM clock gate reaches 8/8 before real work starts.
        nc.gpsimd.memset(warm[:], 0.0)
        for _ in range(24):
            wp = PQ.tile([128, 512], f32, tag="pq", name="warm")
            nc.tensor.matmul(
                wp[:], lhsT=warm[:, 0:128], rhs=warm[:, 128:640],
                start=True, stop=True,
            )

        # contraction chunks in DMA-arrival order
        KORD = [0, 1, 2, 3, 4, 5, 6, 7]

        def proj_unit(w_sb, b_sb, dst, mc, sp):
            # one [128 feats, 512 tokens] projection block: 8 matmuls + bias
            pq = PQ.tile([128, 512], f32, tag="pq")
            for i, k in enumerate(KORD):
                nc.tensor.matmul(
                    pq[:],
                    lhsT=w_sb[:, k, mc * 128 : mc * 128 + 128],
                    rhs=xt[k][:, sp * 512 : sp * 512 + 512],
                    start=(i == 0),
                    stop=(i == 7),
                )
            nc.vector.tensor_scalar_add(
                dst[:, mc, sp * 512 : sp * 512 + 512], pq[:], b_sb[:, mc : mc + 1]
            )

        def v_unit(mt):
            # V token tile [128 tokens, 256 feats] + bias row, scaled by exp(mask)
            pv = PQ.tile([128, 512], f32, tag="pq", name="pv")[:, 0:256]
            for i, k in enumerate(KORD):
                nc.tensor.matmul(
                    pv[:],
                    lhsT=xt[k][:, mt * 128 : mt * 128 + 128],
                    rhs=wv_sb[:, k, :],
                    start=(i == 0),
                    stop=(i == 7),
                )
            nc.vector.tensor_copy(
                v_blk[:, mt, :, 1:65], pv.rearrange("p (l c) -> p l c", l=4)
            )
            nc.vector.tensor_copy(v_blk[:, mt, :, 0], on4_sb[:, 4 * mt : 4 * mt + 4])

        Dps = {}

        def s_kt(p, sp, kt, expP):
            # S^T for key tile kt (both heads of pair p) + mask-free exp,
            # then fold this tile into the DVE running denominator sum.
            qs = sp * 512
            ps = PS.tile([128, 1024], f32, tag="ps")
            for h in range(2):
                rs = 64 * h
                nc.tensor.matmul(
                    ps[:, h * 512 : h * 512 + 512],
                    lhsT=k_sb[rs : rs + 64, p, kt * 128 : kt * 128 + 128],
                    rhs=q_sb[rs : rs + 64, p, qs : qs + 512],
                    start=True,
                    stop=True,
                )
            nc.scalar.activation(
                expP[:, kt, :], ps[:], Exp, bias=mk_sb[:, kt : kt + 1]
            )
            if (p, sp) != (1, 3):
                if kt == 0:
                    Dps[(p, sp)] = dnm.tile([128, 1024], bf16, tag="dp", name="dp")
                    nc.vector.tensor_copy(Dps[(p, sp)][:], expP[:, kt, :])
                else:
                    nc.vector.tensor_add(
                        Dps[(p, sp)][:], Dps[(p, sp)][:], expP[:, kt, :]
                    )

        pcs = {}

        def cu_unit(p, sp, u, expP):
            # 2 col-tiled accumulation steps of ctx^T: both heads of the
            # pair run concurrently in PE column groups (0,0)/(0,64).
            key = (p, sp)
            if u == 0:
                pcs[key] = PC.tile([128, 512], f32, tag="pc", name="pc")
            pc = pcs[key]
            for j in range(2):
                kt = 2 * u + j
                for h in range(2):
                    lh = 2 * p + h
                    nc.tensor.matmul(
                        pc[64 * h : 64 * h + 64, :],
                        lhsT=v_sb[:, kt, 65 * lh + 1 : 65 * lh + 65],
                        rhs=expP[:, kt, h * 512 : h * 512 + 512],
                        start=(kt == 0),
                        stop=(kt == 15),
                        skip_group_check=True,
                    )

        def fin_unit(p, sp):
            # denominator partition-reduce + normalize + store for a
            # col-tiled block: d = ones^T @ Dp, ctx *= 1/d (broadcast).
            qs = sp * 512
            pc = pcs[(p, sp)]
            Dp = Dps[(p, sp)]
            ctxs = nrm.tile([128, 512], f32, tag="cts")
            for h in range(2):
                dps = PQ.tile([128, 512], f32, tag="pq", name="dps")
                nc.tensor.matmul(
                    dps[0:1, :],
                    lhsT=one_sb[:, 0:1],
                    rhs=Dp[:, h * 512 : h * 512 + 512],
                    start=True,
                    stop=True,
                )
                r = nrm.tile([1, 512], f32, tag=f"r{h}")
                nc.vector.reciprocal_approx_fast(r[0:1, :], dps[0:1, :])
                bc = nrm.tile([128, 512], f32, tag=f"bc{h}")
                nc.gpsimd.partition_broadcast(bc[:, :], r[0:1, :])
                nc.vector.tensor_mul(
                    ctxs[64 * h : 64 * h + 64, :],
                    pc[64 * h : 64 * h + 64, :],
                    bc[64 * h : 64 * h + 64, :],
                )
            eng = nc.sync if sp % 2 == 0 else nc.gpsimd
            eng.dma_start(
                out=out_d.ap()[128 * p : 128 * p + 128, qs : qs + 512],
                in_=ctxs[:, :],
            )

        def c65_unit(p, sp, half, u, expP):
            # legacy ctx with in-matmul denominator row (last block only)
            lh = 2 * p + half
            key = (p, sp, half)
            if u == 0:
                pcs[key] = PC.tile([128, 512], f32, tag="pc", name=f"pc65_{half}")
            pc = pcs[key]
            for j in range(2):
                kt = 2 * u + j
                nc.tensor.matmul(
                    pc[0:65, :],
                    lhsT=v_sb[:, kt, 65 * lh : 65 * lh + 65],
                    rhs=expP[:, kt, half * 512 : half * 512 + 512],
                    start=(kt == 0),
                    stop=(kt == 15),
                )
            if u == 7:
                qs = sp * 512
                ctxs = nrm.tile([65, 512], f32, tag="cts65")
                bc = nrm.tile([65, 512], f32, tag="bc65")
                for q in range(2):
                    cs = slice(q * 256, (q + 1) * 256)
                    nc.vector.reciprocal_approx_fast(ctxs[0:1, cs], pc[0:1, cs])
                    nc.gpsimd.partition_broadcast(bc[:, cs], ctxs[0:1, cs])
                    nc.vector.tensor_mul(ctxs[:, cs], pc[0:65, cs], bc[:, cs])
                    eng = nc.sync if q == 0 else nc.gpsimd
                    eng.dma_start(
                        out=out_d.ap()[
                            64 * lh : 64 * lh + 64, qs + cs.start : qs + cs.stop
                        ],
                        in_=ctxs[1:65, cs],
                    )

        # ---- prologue: minimal chain to the first exp
        proj_unit(wq_sb, qb_sb, q_sb, 0, 0)
        proj_unit(wk_sb, kb_sb, k_sb, 0, 0)

        def pq_u(p, sp):
            return ("proj", wq_sb, qb_sb, q_sb, p, sp)

        def pk_u(p, sp):
            return ("proj", wk_sb, kb_sb, k_sb, p, sp)

        # per-block filler schedule: {kt: [tokens emitted after s_kt(kt)]}
        def sched(*pairs):
            d = {}
            for kt, tok in pairs:
                d.setdefault(kt, []).append(tok)
            return d

        fillers = {
            (0, 0): sched(
                (0, pk_u(0, 1)), (4, pk_u(0, 2)), (6, ("v", 0)), (8, pk_u(0, 3)),
                (9, ("v", 1)), (10, ("v", 2)), (12, pq_u(0, 1)), (13, ("v", 3)),
                (14, ("v", 4)),
            ),
            (0, 1): sched(
                (0, ("v", 5)), (1, ("cu", 0, 0, 0)), (2, ("v", 6)),
                (3, ("cu", 0, 0, 1)), (4, ("v", 7)), (5, ("cu", 0, 0, 2)),
                (6, ("v", 8)), (7, ("cu", 0, 0, 3)), (8, ("v", 9)),
                (9, ("cu", 0, 0, 4)), (10, ("v", 10)), (12, ("v", 11)),
                (13, pq_u(0, 2)),
            ),
            (0, 2): sched(
                (0, ("v", 12)), (1, ("v", 13)), (2, ("cu", 0, 0, 5)),
                (3, ("cu", 0, 0, 6)), (4, ("v", 14)), (5, ("v", 15)),
                (6, ("cu", 0, 0, 7)), (7, ("fin", 0, 0)), (8, ("cu", 0, 1, 0)),
                (9, ("cu", 0, 1, 1)), (10, ("cu", 0, 1, 2)), (11, pq_u(0, 3)),
                (12, ("cu", 0, 1, 3)), (13, ("cu", 0, 1, 4)),
                (14, ("cu", 0, 1, 5)), (15, ("cu", 0, 1, 6)),
            ),
            (0, 3): sched(
                (0, ("cu", 0, 1, 7)), (1, ("fin", 0, 1)), (2, ("cu", 0, 2, 0)),
                (3, ("cu", 0, 2, 1)), (4, ("cu", 0, 2, 2)), (5, ("cu", 0, 2, 3)),
                (6, ("cu", 0, 2, 4)), (7, ("cu", 0, 2, 5)), (8, ("cu", 0, 2, 6)),
                (9, ("cu", 0, 2, 7)), (10, ("fin", 0, 2)), (12, pk_u(1, 0)),
                (14, pq_u(1, 0)),
            ),
            (1, 0): sched(
                (0, pk_u(1, 1)), (1, ("cu", 0, 3, 0)), (2, ("cu", 0, 3, 1)),
                (4, pk_u(1, 2)), (5, ("cu", 0, 3, 2)), (6, ("cu", 0, 3, 3)),
                (8, pk_u(1, 3)), (9, ("cu", 0, 3, 4)), (10, ("cu", 0, 3, 5)),
                (12, pq_u(1, 1)),
            ),
            (1, 1): sched(
                (0, ("cu", 0, 3, 6)), (1, ("cu", 0, 3, 7)), (2, ("fin", 0, 3)),
                (3, ("cu", 1, 0, 0)), (4, ("cu", 1, 0, 1)), (5, ("cu", 1, 0, 2)),
                (6, ("cu", 1, 0, 3)), (7, ("cu", 1, 0, 4)), (8, ("cu", 1, 0, 5)),
                (9, ("cu", 1, 0, 6)), (10, ("cu", 1, 0, 7)), (12, ("fin", 1, 0)),
                (13, pq_u(1, 2)),
            ),
            (1, 2): sched(
                (0, ("cu", 1, 1, 0)), (1, ("cu", 1, 1, 1)), (2, pq_u(1, 3)),
                (2, ("cu", 1, 1, 2)), (4, ("cu", 1, 1, 3)), (5, ("cu", 1, 1, 4)),
                (6, ("cu", 1, 1, 5)), (7, ("cu", 1, 1, 6)), (8, ("cu", 1, 1, 7)),
                (9, ("fin", 1, 1)),
            ),
            (1, 3): sched(
                (0, ("cu", 1, 2, 7)), (2, ("fin", 1, 2)),
            ),
        }

        blocks = [(p, sp) for p in (0, 1) for sp in range(4)]
        expPs = {}

        def emit(tok, _ep=None):
            if tok[0] == "v":
                v_unit(tok[1])
            elif tok[0] == "proj":
                _, w, b, dst, mc, sp = tok
                proj_unit(w, b, dst, mc, sp)
            elif tok[0] == "cu":
                _, cp, csp, u = tok
                cu_unit(cp, csp, u, expPs[(cp, csp)])
            elif tok[0] == "fin":
                fin_unit(tok[1], tok[2])

        for p, sp in blocks:
            expP = att.tile([128, 16, 1024], bf16, tag="expP")
            expPs[(p, sp)] = expP
            fill = fillers[(p, sp)]
            chase = (p, sp) in ((1, 2), (1, 3))
            for kt in range(16):
                s_kt(p, sp, kt, expP)
                for tok in fill.get(kt, ()):
                    emit(tok)
                if chase and kt >= 3 and kt % 2 == 1:
                    u = (kt - 3) // 2
                    if (p, sp) == (1, 2):
                        cu_unit(1, 2, u, expP)
                    else:
                        c65_unit(1, 3, 0, u, expP)
                        c65_unit(1, 3, 1, u, expP)

        # epilogue: the last context chunk (its exps just finished)
        for half in range(2):
            c65_unit(1, 3, half, 7, expPs[(1, 3)])

    nc.compile()
    return nc


def _get_program():
    if "nc" not in _CACHE:
        _CACHE["nc"] = _build_program()
    return _CACHE["nc"]


def _to_bf16(x):
    import ml_dtypes

    return np.asarray(x, np.float32).astype(ml_dtypes.bfloat16)


def _make_in_maps(hidden_states, attention_mask, q_w, q_b, k_w, k_b, v_w, v_b):
    hs = np.asarray(hidden_states, np.float32)
    am = np.asarray(attention_mask, np.float32)
    q_w = np.asarray(q_w, np.float32)
    k_w = np.asarray(k_w, np.float32)
    v_w = np.asarray(v_w, np.float32)
    q_b = np.asarray(q_b, np.float32)
    k_b = np.asarray(k_b, np.float32)
    v_b = np.asarray(v_b, np.float32)

    scale = np.float32(1.0 / np.sqrt(HD))
    ones1 = np.ones((128, 1), np.float32)

    in_maps = []
    for c in range(NCORES):
        b = c // 4
        hg = c % 4
        cols = slice(WCOLS * hg, WCOLS * hg + WCOLS)
        mask = am[b, 0, 0, :]  # [S]
        mk = np.ascontiguousarray(mask.reshape(16, 128).T.astype(np.float32))
        # xt: [1024, 2048] -> [quarter, chunk, 128, 512]
        xt = hs[b].T.reshape(8, 128, 4, 512).transpose(2, 0, 1, 3)

        def wlay(w):
            # [1024, 256] -> [mc, 128, 8, 128] (pair-major, partition-major)
            return np.ascontiguousarray(
                _to_bf16(
                    w.reshape(8, 128, 2, 128).transpose(2, 1, 0, 3)
                )
            )

        in_maps.append(
            {
                "xt": np.ascontiguousarray(_to_bf16(xt)),
                "wq": wlay(q_w[:, cols] * scale),
                "wk": wlay(k_w[:, cols]),
                "wv": np.ascontiguousarray(
                    _to_bf16(v_w[:, cols].reshape(8, 128, WCOLS).transpose(1, 0, 2))
                ),
                "qb2": np.ascontiguousarray((q_b[cols] * scale).reshape(2, 128).T),
                "kb2": np.ascontiguousarray(k_b[cols].reshape(2, 128).T),
                "mk": mk,
                "ones4": np.ascontiguousarray(_to_bf16(np.ones((128, 64)))),
                "one1": np.ascontiguousarray(_to_bf16(ones1)),
            }
        )
    return in_maps


def kernel(hidden_states, attention_mask, q_w, q_b, k_w, k_b, v_w, v_b):
    from concourse import bass_utils

    nc = _get_program()
    in_maps = _make_in_maps(
        hidden_states, attention_mask, q_w, q_b, k_w, k_b, v_w, v_b
    )
    _CACHE["in_maps"] = in_maps
    res = bass_utils.run_bass_kernel_spmd(nc, in_maps, core_ids=list(range(NCORES)))

    full = np.empty((B, S, HIDDEN), np.float32)
    for c in range(NCORES):
        b = c // 4
        hg = c % 4
        full[b, :, WCOLS * hg : WCOLS * hg + WCOLS] = res.results[c]["out"].T
    # V bias contributes exactly v_b to every context vector (softmax
    # weights sum to 1), so it is added here instead of on-device.
    full += np.asarray(v_b, np.float32)[None, None, :]
    return full


# revision 10
# speedup vs baseline: 1.1690x; 1.0872x over previous
"""BertSelfAttention on 8 Trainium2 NeuronCores.

Sharding: data parallel over batch (B=2) x tensor parallel over heads
(16 heads -> 4 groups of 4). Core c handles batch c//4, heads 4*(c%4)..+4.
No collectives: each core produces a disjoint [256, 2048] output slice
(feature-major); the host transposes/concatenates.

Per-core program (identical on all cores, SPMD over data):
  inputs (host-prepped):
    xt    [2, 8, 128, 1024]  hidden_states[b].T, split [half, chunk] (bf16)
    wq/wk/wv [128, 8, 256] weight column slices, partition-major, bf16
             (wq,qb2 pre-scaled 1/8)
    qb2/kb2 [128, 2]    bias chunks (per-partition layout, f32)
    em    [128, 16]     exp(mask) chunks (f32), emb same in bf16
    em4   [128, 64]     exp(mask) x4 replicated (bf16)
    one1  [128, 1]      ones (bf16)
  output:
    out   [256, 2048]   context slice, feature-major (f32)

The attention mask is folded into V: softmax(s+m) @ V == (exp(s) @
diag(exp(m)) V) / (sum_k exp(s) exp(m)), so V rows are pre-scaled by
exp(mask) and the exp on the Activation engine needs no bias operand.

Design (trace-driven):
- ACT (exp) is the roofline: 128 x [128,1024] exps at ~1.12us = 143us.
  Span ~= first-exp time + ACT stream + tail, so the prologue streams
  inputs over 5 DMA queues (contiguous host layouts) and ~8 junk
  matmuls warm the PE HAM clock gate during the DMA wait, putting the
  first exp at ~8us instead of 28us.
- Context matmuls are column-tiled: two heads' [64,512] ctx tiles run
  concurrently in PE column groups (0,0)/(0,64), halving ctx PE time so
  total PE work sits ~20us under the ACT roofline and the exp stream
  never starves. The softmax denominators that previously rode the ctx
  matmul as a 65th row come from a DVE running sum over key tiles
  (bf16) finished by a tiny ones-column matmul partition reduction.
- The last block (1,3) keeps the legacy 65-row ctx (denominator
  in-matmul) so the critical tail after the final exp is short.
"""

import numpy as np

HIDDEN = 1024
HEADS = 16
HD = 64
B = 2
S = 2048
NCORES = 8
HPC = HEADS // 4  # heads per core = 4
WCOLS = HPC * HD  # 256 weight columns per core

_CACHE = {}


def _build_program():
    import concourse.bass as bass
    import concourse.bacc as bacc
    import concourse.tile as tile
    import concourse.mybir as mybir

    f32 = mybir.dt.float32
    bf16 = mybir.dt.bfloat16
    Exp = mybir.ActivationFunctionType.Exp
    MUL = mybir.AluOpType.mult
    ADD = mybir.AluOpType.add

    nc = bacc.Bacc("TRN2", target_bir_lowering=False, debug=False, num_devices=NCORES)

    xt_d = nc.dram_tensor("xt", [4, 8, 128, 512], bf16, kind="ExternalInput")
    wq_d = nc.dram_tensor("wq", [2, 128, 8, 128], bf16, kind="ExternalInput")
    wk_d = nc.dram_tensor("wk", [2, 128, 8, 128], bf16, kind="ExternalInput")
    wv_d = nc.dram_tensor("wv", [128, 8, WCOLS], bf16, kind="ExternalInput")
    qb_d = nc.dram_tensor("qb2", [128, 2], f32, kind="ExternalInput")
    kb_d = nc.dram_tensor("kb2", [128, 2], f32, kind="ExternalInput")
    mk_d = nc.dram_tensor("mk", [128, 16], f32, kind="ExternalInput")
    on4_d = nc.dram_tensor("ones4", [128, 64], bf16, kind="ExternalInput")
    one_d = nc.dram_tensor("one1", [128, 1], bf16, kind="ExternalInput")
    out_d = nc.dram_tensor("out", [WCOLS, S], f32, kind="ExternalOutput")

    with (
        tile.TileContext(nc) as tc,
        tc.tile_pool(name="main", bufs=1) as P,
        tc.tile_pool(name="att", bufs=3) as att,
        tc.tile_pool(name="nrm", bufs=2) as nrm,
        tc.tile_pool(name="dnm", bufs=2) as dnm,
        tc.tile_pool(name="pqp", bufs=1, space="PSUM") as PQ,
        tc.tile_pool(name="psp", bufs=2, space="PSUM") as PS,
        tc.tile_pool(name="pcp", bufs=3, space="PSUM") as PC,
    ):
        xtb = P.tile([128, 8, S], bf16)
        wq_sb = P.tile([128, 8, WCOLS], bf16)
        wk_sb = P.tile([128, 8, WCOLS], bf16)
        wv_sb = P.tile([128, 8, WCOLS], bf16)
        q_sb = P.tile([128, 2, S], bf16)  # [feat(2 heads), pair, token]
        k_sb = P.tile([128, 2, S], bf16)
        v_sb = P.tile([128, 16, 4 * 65], bf16)  # [token, tile, 4*(em + 64 feats)]
        v_blk = v_sb.rearrange("p m (l c) -> p m l c", l=4)
        qkb = P.tile([128, 4], f32)
        qb_sb = qkb[:, 0:2]
        kb_sb = qkb[:, 2:4]
        mk_sb = P.tile([128, 16], f32)
        on4_sb = P.tile([128, 64], bf16)
        one_sb = P.tile([128, 1], bf16)
        warm = P.tile([128, 640], bf16)

        # ---- memset first so the PE warmup (which reads `warm`) can
        # start immediately, before any DMA trigger occupies gpsimd.
        nc.gpsimd.memset(warm[:], 0.0)

        # ---- input DMAs on the 3 HW queues (sync, gpsimd, scalar).
        # Aggregate DMA BW is ~350GB/s with ~9us startup; each wave is
        # split evenly across the 3 queues so it lands as early as
        # possible. Wave 1 (wq/wk pair-0 halves + xt token-quarter 0,
        # 1.5MB) is all the first exp needs. Scalar-queue triggers all
        # execute on ACT at t<2us, long before the first exp.
        def xq(eng, q, c0, c1):
            eng.dma_start(
                out=xtb[:, c0:c1, 512 * q : 512 * q + 512],
                in_=xt_d.ap()[q, c0:c1].rearrange("c p t -> p c t"),
            )

        nc.sync.dma_start(out=qb_sb, in_=qb_d.ap())
        nc.sync.dma_start(out=kb_sb, in_=kb_d.ap())
        nc.sync.dma_start(out=mk_sb[:], in_=mk_d.ap())
        # wave 1
        nc.sync.dma_start(out=wq_sb[:, :, 0:128], in_=wq_d.ap()[0])
        nc.gpsimd.dma_start(out=wk_sb[:, :, 0:128], in_=wk_d.ap()[0])
        xq(nc.sync, 0, 0, 2)
        xq(nc.gpsimd, 0, 2, 4)
        xq(nc.scalar, 0, 4, 8)
        # wave 2: xt quarter 1 (keys 512-1023 needed by s4)
        xq(nc.sync, 1, 0, 3)
        xq(nc.gpsimd, 1, 3, 6)
        xq(nc.scalar, 1, 6, 8)
        # wave 3: xt quarter 2
        xq(nc.sync, 2, 0, 3)
        xq(nc.gpsimd, 2, 3, 6)
        xq(nc.scalar, 2, 6, 8)
        # wave 4: V weights + constants
        nc.sync.dma_start(out=wv_sb[:, 0:4, :], in_=wv_d.ap()[:, 0:4, :])
        nc.gpsimd.dma_start(out=wv_sb[:, 4:8, :], in_=wv_d.ap()[:, 4:8, :])
        nc.gpsimd.dma_start(out=on4_sb[:], in_=on4_d.ap())
        nc.gpsimd.dma_start(out=one_sb[:], in_=one_d.ap())
        # wave 5: xt quarter 3
        xq(nc.sync, 3, 0, 4)
        xq(nc.gpsimd, 3, 4, 8)
        # wave 6: mc1 weight halves (first needed ~70us in)
        nc.sync.dma_start(out=wq_sb[:, :, 128:256], in_=wq_d.ap()[1])
        nc.gpsimd.dma_start(out=wk_sb[:, :, 128:256], in_=wk_d.ap()[1])

        # ---- PE warmup: WAW-chained junk matmuls (~640ns each cold)
        # bridge the ~14us DMA wait and keep the HAM clock gate at 8/8.
        for _ in range(24):
            wp = PQ.tile([128, 512], f32, tag="pq", name="warm")
            nc.tensor.matmul(
                wp[:], lhsT=warm[:, 0:128], rhs=warm[:, 128:640],
                start=True, stop=True,
            )

        # contraction chunks in DMA-arrival order
        KORD = [0, 1, 2, 3, 4, 5, 6, 7]

        def proj_unit(w_sb, b_sb, dst, mc, sp):
            # one [128 feats, 512 tokens] projection block: 8 matmuls + bias
            pq = PQ.tile([128, 512], f32, tag="pq")
            for i, k in enumerate(KORD):
                nc.tensor.matmul(
                    pq[:],
                    lhsT=w_sb[:, k, mc * 128 : mc * 128 + 128],
                    rhs=xtb[:, k, sp * 512 : sp * 512 + 512],
                    start=(i == 0),
                    stop=(i == 7),
                )
            nc.vector.tensor_scalar_add(
                dst[:, mc, sp * 512 : sp * 512 + 512], pq[:], b_sb[:, mc : mc + 1]
            )

        def v_unit(mt):
            # V token tile [128 tokens, 256 feats] + bias row, scaled by exp(mask)
            pv = PQ.tile([128, 512], f32, tag="pq", name="pv")[:, 0:256]
            for i, k in enumerate(KORD):
                nc.tensor.matmul(
                    pv[:],
                    lhsT=xtb[:, k, mt * 128 : mt * 128 + 128],
                    rhs=wv_sb[:, k, :],
                    start=(i == 0),
                    stop=(i == 7),
                )
            nc.vector.tensor_copy(
                v_blk[:, mt, :, 1:65], pv.rearrange("p (l c) -> p l c", l=4)
            )
            nc.vector.tensor_copy(v_blk[:, mt, :, 0], on4_sb[:, 4 * mt : 4 * mt + 4])

        Dps = {}

        def s_kt(p, sp, kt, expP):
            # S^T for key tile kt (both heads of pair p) + mask-free exp,
            # then fold this tile into the DVE running denominator sum.
            qs = sp * 512
            ps = PS.tile([128, 1024], f32, tag="ps")
            for h in range(2):
                rs = 64 * h
                nc.tensor.matmul(
                    ps[:, h * 512 : h * 512 + 512],
                    lhsT=k_sb[rs : rs + 64, p, kt * 128 : kt * 128 + 128],
                    rhs=q_sb[rs : rs + 64, p, qs : qs + 512],
                    start=True,
                    stop=True,
                )
            nc.scalar.activation(
                expP[:, kt, :], ps[:], Exp, bias=mk_sb[:, kt : kt + 1]
            )
            if (p, sp) != (1, 3):
                if kt == 0:
                    Dps[(p, sp)] = dnm.tile([128, 1024], bf16, tag="dp", name="dp")
                    nc.vector.tensor_copy(Dps[(p, sp)][:], expP[:, kt, :])
                else:
                    nc.vector.tensor_add(
                        Dps[(p, sp)][:], Dps[(p, sp)][:], expP[:, kt, :]
                    )

        pcs = {}

        def cu_unit(p, sp, u, expP):
            # 2 col-tiled accumulation steps of ctx^T: both heads of the
            # pair run concurrently in PE column groups (0,0)/(0,64).
            key = (p, sp)
            if u == 0:
                pcs[key] = PC.tile([128, 512], f32, tag="pc", name="pc")
            pc = pcs[key]
            for j in range(2):
                kt = 2 * u + j
                for h in range(2):
                    lh = 2 * p + h
                    nc.tensor.matmul(
                        pc[64 * h : 64 * h + 64, :],
                        lhsT=v_sb[:, kt, 65 * lh + 1 : 65 * lh + 65],
                        rhs=expP[:, kt, h * 512 : h * 512 + 512],
                        start=(kt == 0),
                        stop=(kt == 15),
                        skip_group_check=True,
                    )

        def fin_unit(p, sp):
            # denominator partition-reduce + normalize + store for a
            # col-tiled block: d = ones^T @ Dp, ctx *= 1/d (broadcast).
            qs = sp * 512
            pc = pcs[(p, sp)]
            Dp = Dps[(p, sp)]
            ctxs = nrm.tile([128, 512], f32, tag="cts")
            for h in range(2):
                dps = PQ.tile([128, 512], f32, tag="pq", name="dps")
                nc.tensor.matmul(
                    dps[0:1, :],
                    lhsT=one_sb[:, 0:1],
                    rhs=Dp[:, h * 512 : h * 512 + 512],
                    start=True,
                    stop=True,
                )
                r = nrm.tile([1, 512], f32, tag=f"r{h}")
                nc.vector.reciprocal_approx_fast(r[0:1, :], dps[0:1, :])
                bc = nrm.tile([128, 512], f32, tag=f"bc{h}")
                nc.gpsimd.partition_broadcast(bc[:, :], r[0:1, :])
                nc.vector.tensor_mul(
                    ctxs[64 * h : 64 * h + 64, :],
                    pc[64 * h : 64 * h + 64, :],
                    bc[64 * h : 64 * h + 64, :],
                )
            eng = nc.sync if sp % 2 == 0 else nc.gpsimd
            eng.dma_start(
                out=out_d.ap()[128 * p : 128 * p + 128, qs : qs + 512],
                in_=ctxs[:, :],
            )

        def c65_unit(p, sp, half, u, expP):
            # legacy ctx with in-matmul denominator row (last block only)
            lh = 2 * p + half
            key = (p, sp, half)
            if u == 0:
                pcs[key] = PC.tile([128, 512], f32, tag="pc", name=f"pc65_{half}")
            pc = pcs[key]
            for j in range(2):
                kt = 2 * u + j
                nc.tensor.matmul(
                    pc[0:65, :],
                    lhsT=v_sb[:, kt, 65 * lh : 65 * lh + 65],
                    rhs=expP[:, kt, half * 512 : half * 512 + 512],
                    start=(kt == 0),
                    stop=(kt == 15),
                )
            if u == 7:
                qs = sp * 512
                ctxs = nrm.tile([65, 512], f32, tag="cts65")
                bc = nrm.tile([65, 512], f32, tag="bc65")
                for q in range(2):
                    cs = slice(q * 256, (q + 1) * 256)
                    nc.vector.reciprocal_approx_fast(ctxs[0:1, cs], pc[0:1, cs])
                    nc.gpsimd.partition_broadcast(bc[:, cs], ctxs[0:1, cs])
                    nc.vector.tensor_mul(ctxs[:, cs], pc[0:65, cs], bc[:, cs])
                    eng = nc.sync if q == 0 else nc.gpsimd
                    eng.dma_start(
                        out=out_d.ap()[
                            64 * lh : 64 * lh + 64, qs + cs.start : qs + cs.stop
                        ],
                        in_=ctxs[1:65, cs],
                    )

        # ---- prologue: minimal chain to the first exp
        proj_unit(wq_sb, qb_sb, q_sb, 0, 0)
        proj_unit(wk_sb, kb_sb, k_sb, 0, 0)

        def pq_u(p, sp):
            return ("proj", wq_sb, qb_sb, q_sb, p, sp)

        def pk_u(p, sp):
            return ("proj", wk_sb, kb_sb, k_sb, p, sp)

        # per-block filler schedule: {kt: [tokens emitted after s_kt(kt)]}
        def sched(*pairs):
            d = {}
            for kt, tok in pairs:
                d.setdefault(kt, []).append(tok)
            return d

        fillers = {
            (0, 0): sched(
                (0, pk_u(0, 1)), (4, pk_u(0, 2)), (6, ("v", 0)), (8, pk_u(0, 3)),
                (9, ("v", 1)), (10, ("v", 2)), (12, pq_u(0, 1)), (13, ("v", 3)),
                (14, ("v", 4)),
            ),
            (0, 1): sched(
                (0, ("v", 5)), (1, ("cu", 0, 0, 0)), (2, ("v", 6)),
                (3, ("cu", 0, 0, 1)), (4, ("v", 7)), (5, ("cu", 0, 0, 2)),
                (6, ("v", 8)), (7, ("cu", 0, 0, 3)), (8, ("v", 9)),
                (9, ("cu", 0, 0, 4)), (10, ("v", 10)), (12, ("v", 11)),
                (13, pq_u(0, 2)),
            ),
            (0, 2): sched(
                (0, ("v", 12)), (1, ("v", 13)), (2, ("cu", 0, 0, 5)),
                (3, ("cu", 0, 0, 6)), (4, ("v", 14)), (5, ("v", 15)),
                (6, ("cu", 0, 0, 7)), (7, ("fin", 0, 0)), (8, ("cu", 0, 1, 0)),
                (9, ("cu", 0, 1, 1)), (10, ("cu", 0, 1, 2)), (11, pq_u(0, 3)),
                (12, ("cu", 0, 1, 3)), (13, ("cu", 0, 1, 4)),
                (14, ("cu", 0, 1, 5)), (15, ("cu", 0, 1, 6)),
            ),
            (0, 3): sched(
                (0, ("cu", 0, 1, 7)), (1, ("fin", 0, 1)), (2, ("cu", 0, 2, 0)),
                (3, ("cu", 0, 2, 1)), (4, ("cu", 0, 2, 2)), (5, ("cu", 0, 2, 3)),
                (6, ("cu", 0, 2, 4)), (7, ("cu", 0, 2, 5)), (8, ("cu", 0, 2, 6)),
                (9, ("cu", 0, 2, 7)), (10, ("fin", 0, 2)), (12, pk_u(1, 0)),
                (14, pq_u(1, 0)),
            ),
            (1, 0): sched(
                (0, pk_u(1, 1)), (1, ("cu", 0, 3, 0)), (2, ("cu", 0, 3, 1)),
                (4, pk_u(1, 2)), (5, ("cu", 0, 3, 2)), (6, ("cu", 0, 3, 3)),
                (8, pk_u(1, 3)), (9, ("cu", 0, 3, 4)), (10, ("cu", 0, 3, 5)),
                (12, pq_u(1, 1)),
            ),
            (1, 1): sched(
                (0, ("cu", 0, 3, 6)), (1, ("cu", 0, 3, 7)), (2, ("fin", 0, 3)),
                (3, ("cu", 1, 0, 0)), (4, ("cu", 1, 0, 1)), (5, ("cu", 1, 0, 2)),
                (6, ("cu", 1, 0, 3)), (7, ("cu", 1, 0, 4)), (8, ("cu", 1, 0, 5)),
                (9, ("cu", 1, 0, 6)), (10, ("cu", 1, 0, 7)), (12, ("fin", 1, 0)),
                (13, pq_u(1, 2)),
            ),
            (1, 2): sched(
                (0, ("cu", 1, 1, 0)), (1, ("cu", 1, 1, 1)), (2, pq_u(1, 3)),
                (2, ("cu", 1, 1, 2)), (4, ("cu", 1, 1, 3)), (5, ("cu", 1, 1, 4)),
                (6, ("cu", 1, 1, 5)), (7, ("cu", 1, 1, 6)), (8, ("cu", 1, 1, 7)),
                (9, ("fin", 1, 1)),
            ),
            (1, 3): sched(
                (0, ("cu", 1, 2, 7)), (2, ("fin", 1, 2)),
            ),
        }

        blocks = [(p, sp) for p in (0, 1) for sp in range(4)]
        expPs = {}

        def emit(tok, _ep=None):
            if tok[0] == "v":
                v_unit(tok[1])
            elif tok[0] == "proj":
                _, w, b, dst, mc, sp = tok
                proj_unit(w, b, dst, mc, sp)
            elif tok[0] == "cu":
                _, cp, csp, u = tok
                cu_unit(cp, csp, u, expPs[(cp, csp)])
            elif tok[0] == "fin":
                fin_unit(tok[1], tok[2])

        for p, sp in blocks:
            expP = att.tile([128, 16, 1024], bf16, tag="expP")
            expPs[(p, sp)] = expP
            fill = fillers[(p, sp)]
            chase = (p, sp) in ((1, 2), (1, 3))
            for kt in range(16):
                s_kt(p, sp, kt, expP)
                for tok in fill.get(kt, ()):
                    emit(tok)
                if chase and kt >= 3 and kt % 2 == 1:
                    u = (kt - 3) // 2
                    if (p, sp) == (1, 2):
                        cu_unit(1, 2, u, expP)
                    else:
                        c65_unit(1, 3, 0, u, expP)
                        c65_unit(1, 3, 1, u, expP)

        # epilogue: the last context chunk (its exps just finished)
        for half in range(2):
            c65_unit(1, 3, half, 7, expPs[(1, 3)])

    nc.compile()
    return nc


def _get_program():
    if "nc" not in _CACHE:
        _CACHE["nc"] = _build_program()
    return _CACHE["nc"]


def _to_bf16(x):
    import ml_dtypes

    return np.asarray(x, np.float32).astype(ml_dtypes.bfloat16)


def _make_in_maps(hidden_states, attention_mask, q_w, q_b, k_w, k_b, v_w, v_b):
    hs = np.asarray(hidden_states, np.float32)
    am = np.asarray(attention_mask, np.float32)
    q_w = np.asarray(q_w, np.float32)
    k_w = np.asarray(k_w, np.float32)
    v_w = np.asarray(v_w, np.float32)
    q_b = np.asarray(q_b, np.float32)
    k_b = np.asarray(k_b, np.float32)
    v_b = np.asarray(v_b, np.float32)

    scale = np.float32(1.0 / np.sqrt(HD))
    ones1 = np.ones((128, 1), np.float32)

    in_maps = []
    for c in range(NCORES):
        b = c // 4
        hg = c % 4
        cols = slice(WCOLS * hg, WCOLS * hg + WCOLS)
        mask = am[b, 0, 0, :]  # [S]
        mk = np.ascontiguousarray(mask.reshape(16, 128).T.astype(np.float32))
        # xt: [1024, 2048] -> [quarter, chunk, 128, 512]
        xt = hs[b].T.reshape(8, 128, 4, 512).transpose(2, 0, 1, 3)

        def wlay(w):
            # [1024, 256] -> [mc, 128, 8, 128] (pair-major, partition-major)
            return np.ascontiguousarray(
                _to_bf16(
                    w.reshape(8, 128, 2, 128).transpose(2, 1, 0, 3)
                )
            )

        in_maps.append(
            {
                "xt": np.ascontiguousarray(_to_bf16(xt)),
                "wq": wlay(q_w[:, cols] * scale),
                "wk": wlay(k_w[:, cols]),
                "wv": np.ascontiguousarray(
                    _to_bf16(v_w[:, cols].reshape(8, 128, WCOLS).transpose(1, 0, 2))
                ),
                "qb2": np.ascontiguousarray((q_b[cols] * scale).reshape(2, 128).T),
                "kb2": np.ascontiguousarray(k_b[cols].reshape(2, 128).T),
                "mk": mk,
                "ones4": np.ascontiguousarray(_to_bf16(np.ones((128, 64)))),
                "one1": np.ascontiguousarray(_to_bf16(ones1)),
            }
        )
    return in_maps


def kernel(hidden_states, attention_mask, q_w, q_b, k_w, k_b, v_w, v_b):
    from concourse import bass_utils

    nc = _get_program()
    in_maps = _make_in_maps(
        hidden_states, attention_mask, q_w, q_b, k_w, k_b, v_w, v_b
    )
    _CACHE["in_maps"] = in_maps
    res = bass_utils.run_bass_kernel_spmd(nc, in_maps, core_ids=list(range(NCORES)))

    full = np.empty((B, S, HIDDEN), np.float32)
    for c in range(NCORES):
        b = c // 4
        hg = c % 4
        full[b, :, WCOLS * hg : WCOLS * hg + WCOLS] = res.results[c]["out"].T
    # V bias contributes exactly v_b to every context vector (softmax
    # weights sum to 1), so it is added here instead of on-device.
    full += np.asarray(v_b, np.float32)[None, None, :]
    return full
